# revision 2
# baseline (speedup 1.0000x reference)
"""Beran estimator kernel for Trainium2 (8 NeuronCores, data-parallel over batch).

Math (per test sample b over N train points):
  p = softmax(c_p)                                  [K, V] per b
  T(n)  = sum_k p[k, c_in[n,k]]                     (gather-sum == one-hot matmul)
  metric= sum_k sumsq_k + K - 2 T
  w     = exp(-metric/bw)          (unnormalized; normalizer s folds away)
  Rinc(n) = sum_{m>=n} w(m)        (suffix cumsum; == s*(1-shifted))
  Rexc(n) = Rinc(n) - w(n)         (== s*(1-wc))
  bad(n)  = Rexc(n) <= 1.00001e-5 * s     (== isclose union, since Rexc<=Rinc)
  xi    = (ln Rinc - ln Rexc) * !bad * delta
  H     = forward cumsum(xi);  surv = exp(-H)
  steps = surv * (exp(xi_eff) - 1);  steps /= (1 - surv_last)

Device layout: partitions = (b in 0..64) x (n-half), free = 2048 n's.
"""

import sys

sys.path.insert(0, "/opt/trn_rl_repo")

import numpy as np

K, B, N, V = 16, 512, 4096, 32
NCORES = 8
BL = B // NCORES  # 64 test samples per core
NH = N // 2  # 2048, free-dim length per half
EPS = 1e-13
ISO = 1.00001e-5  # atol + rtol*|1| of np.isclose(x, 1.0)

_cache = {}


def _build_nc():
    import concourse.bacc as bacc
    import concourse.mybir as mybir
    from concourse import tile
    from contextlib import ExitStack

    dt = mybir.dt
    f32, bf16 = dt.float32, dt.bfloat16
    Alu = mybir.AluOpType
    Act = mybir.ActivationFunctionType

    nc = bacc.Bacc("TRN2", target_bir_lowering=False, debug=False, num_devices=NCORES)

    cp2 = nc.dram_tensor("cp2", [128, 512], f32, kind="ExternalInput").ap()
    ohot = nc.dram_tensor("ohot", [512, N], bf16, kind="ExternalInput").ap()
    deltab = nc.dram_tensor("deltab", [128, NH], bf16, kind="ExternalInput").ap()
    mrb = nc.dram_tensor("mrb", [128, 1], f32, kind="ExternalInput").ap()
    scl2 = nc.dram_tensor("scl2", [128, 1], f32, kind="ExternalInput").ap()
    idn = nc.dram_tensor("idn", [64, 64], f32, kind="ExternalInput").ap()
    surv_o = nc.dram_tensor("surv_o", [BL, N], f32, kind="ExternalOutput").ap()
    steps_o = nc.dram_tensor("steps_o", [BL, N], f32, kind="ExternalOutput").ap()

    with ExitStack() as ctx:
        tc = ctx.enter_context(tile.TileContext(nc))
        sb = ctx.enter_context(tc.tile_pool(name="sb", bufs=1))
        pp = ctx.enter_context(tc.tile_pool(name="pp", bufs=1, space="PSUM"))

        # ---- input DMAs ----
        cp_s = sb.tile([128, 512], f32)
        nc.sync.dma_start(cp_s[:], cp2)
        ohot_s = []
        for c in range(4):
            t = sb.tile([128, N], bf16, name=f"ohot_s{c}")
            nc.sync.dma_start(t[:], ohot[128 * c : 128 * (c + 1), :])
            ohot_s.append(t)
        deltab_s = sb.tile([128, NH], bf16)
        nc.sync.dma_start(deltab_s[:], deltab)
        mrb_s = sb.tile([128, 1], f32)
        nc.sync.dma_start(mrb_s[:], mrb)
        scl2_s = sb.tile([128, 1], f32)
        nc.sync.dma_start(scl2_s[:], scl2)
        idn_s = sb.tile([64, 64], f32)
        nc.sync.dma_start(idn_s[:], idn)

        # ---- softmax over v (free-dim segments of 32), rows duplicated ----
        cp3 = cp_s[:].rearrange("p (k v) -> p k v", v=V)
        mx = sb.tile([128, K], f32)
        nc.vector.tensor_reduce(mx[:], cp3, axis=mybir.AxisListType.X, op=Alu.max)
        mx3 = mx[:].unsqueeze(2).broadcast_to([128, K, V])
        e3t = sb.tile([128, 512], f32, name="e3t")
        e3 = e3t[:].rearrange("p (k v) -> p k v", v=V)
        nc.vector.tensor_tensor(e3, cp3, mx3, op=Alu.subtract)
        nc.scalar.activation(e3t[:], e3t[:], Act.Exp)
        sm = sb.tile([128, K], f32)
        nc.vector.tensor_reduce(sm[:], e3, axis=mybir.AxisListType.X, op=Alu.add)
        rsm = sb.tile([128, K], f32)
        nc.vector.reciprocal(rsm[:], sm[:])
        rsm3 = rsm[:].unsqueeze(2).broadcast_to([128, K, V])
        p_t = sb.tile([128, 512], f32, name="p_t")
        p3 = p_t[:].rearrange("p (k v) -> p k v", v=V)
        nc.vector.tensor_tensor(p3, e3, rsm3, op=Alu.mult)
        # Ssum = sum p^2
        psq = sb.tile([128, 512], f32, name="psq")
        nc.vector.tensor_tensor(psq[:], p_t[:], p_t[:], op=Alu.mult)
        ssum = sb.tile([128, 1], f32)
        nc.vector.tensor_reduce(ssum[:], psq[:], axis=mybir.AxisListType.X, op=Alu.add)
        # biasw = (Ssum + K) * (-1/bw)
        biasw = sb.tile([128, 1], f32)
        nc.vector.tensor_scalar(biasw[:], ssum[:], float(K), mrb_s[:], op0=Alu.add, op1=Alu.mult)

        # ---- transpose p (rows 0:64) into [kv, b]; split hi/lo bf16 ----
        rhs_pair = []
        for c in range(4):
            tp = pp.tile([128, 64], f32, name=f"tp{c}")
            nc.tensor.transpose(tp[:], p_t[0:64, 128 * c : 128 * (c + 1)], idn_s[:])
            rp = sb.tile([128, 128], bf16, name=f"rp{c}")
            nc.vector.tensor_copy(rp[:, 0:64], tp[:])
            nc.vector.tensor_tensor(rp[:, 64:128], tp[:], rp[:, 0:64], op=Alu.subtract)
            rhs_pair.append(rp)

        # ---- G matmuls: psum pair j rows0:64 = n-chunk j, rows64:128 = chunk j+4
        w_all = sb.tile([128, NH], f32)
        for j in range(4):
            tps = pp.tile([128, 512], f32, name=f"tps{j}")
            for half in range(2):
                out_ap = tps[64 * half : 64 * half + 64, :]
                ns = 512 * (j + 4 * half)
                mm = 0
                for c in range(4):
                    for hl in range(2):
                        nc.tensor.matmul(
                            out_ap,
                            lhsT=rhs_pair[c][:, 64 * hl : 64 * hl + 64],
                            rhs=ohot_s[c][:, ns : ns + 512],
                            start=(mm == 0),
                            stop=(mm == 7),
                        )
                        mm += 1
            # w = exp(scl2 * T + biasw)
            nc.scalar.activation(
                w_all[:, 512 * j : 512 * (j + 1)], tps[:], Act.Exp,
                bias=biasw[:], scale=scl2_s[:],
            )

        # ---- suffix scan (reversed free dim): Rloc = local inclusive suffix sum
        rloc = sb.tile([128, NH], f32)
        w_rev = w_all[:, ::-1]
        r_rev = rloc[:, ::-1]
        nc.vector.tensor_tensor_scan(
            r_rev, w_rev, w_rev, initial=0.0, op0=Alu.add, op1=Alu.bypass
        )

        # carry for lower rows = upper-half totals (Rloc[64:128, 0])
        carry = sb.tile([128, 1], f32)
        nc.vector.memset(carry[:], 0.0)
        nc.sync.dma_start(carry[0:64, :], rloc[64:128, 0:1])

        # s = Rloc[0:64,0] + carry ; ts = ISO*s (+1e30 where s < EPS)
        scol = sb.tile([128, 1], f32)  # rows 0:64 meaningful
        nc.vector.tensor_scalar(scol[0:64, :], rloc[0:64, 0:1], carry[0:64, :], None, op0=Alu.add)
        tsg = sb.tile([128, 1], f32)
        nc.vector.tensor_scalar(tsg[0:64, :], scol[0:64, :], ISO, None, op0=Alu.mult)
        gcol = sb.tile([128, 1], f32)
        nc.vector.tensor_scalar(gcol[0:64, :], scol[0:64, :], EPS, None, op0=Alu.is_lt)
        tsf = sb.tile([128, 1], f32)
        nc.vector.scalar_tensor_tensor(
            tsf[0:64, :], gcol[0:64, :], 1e30, tsg[0:64, :], op0=Alu.mult, op1=Alu.add
        )
        nc.sync.dma_start(tsf[64:128, :], tsf[0:64, :])
        # threshold adjusted for local (carry-less) compare: ts - carry
        tsadj = sb.tile([128, 1], f32)
        nc.vector.tensor_tensor(tsadj[:], tsf[:], carry[:], op=Alu.subtract)

        # Rexc_loc = Rloc - w
        rexc = sb.tile([128, NH], f32)
        nc.vector.tensor_tensor(rexc[:], rloc[:], w_all[:], op=Alu.subtract)

        # gd = (Rexc_loc > ts - carry) * delta
        gd = sb.tile([128, NH], bf16)
        nc.vector.scalar_tensor_tensor(
            gd[:], rexc[:], tsadj[:], deltab_s[:], op0=Alu.is_gt, op1=Alu.mult
        )

        # xi = (ln(Rloc + carry + eps) - ln(Rexc_loc + carry + eps)) * gd
        carrye = sb.tile([128, 1], f32)
        nc.vector.tensor_scalar(carrye[:], carry[:], 1e-30, None, op0=Alu.add)
        lnri = sb.tile([128, NH], f32)
        nc.scalar.activation(lnri[:], rloc[:], Act.Ln, bias=carrye[:])
        lnre = sb.tile([128, NH], f32)
        nc.scalar.activation(lnre[:], rexc[:], Act.Ln, bias=carrye[:])
        xi0 = sb.tile([128, NH], f32)
        nc.vector.tensor_tensor(xi0[:], lnri[:], lnre[:], op=Alu.subtract)
        xie = sb.tile([128, NH], f32)
        nc.vector.tensor_tensor(xie[:], xi0[:], gd[:], op=Alu.mult)

        # forward scan: Hloc
        hloc = sb.tile([128, NH], f32)
        nc.vector.tensor_tensor_scan(
            hloc[:], xie[:], xie[:], initial=0.0, op0=Alu.add, op1=Alu.bypass
        )
        # carryH for upper rows = Hloc[0:64, last]
        mch = sb.tile([128, 1], f32)
        nc.vector.memset(mch[:], 0.0)
        nc.sync.dma_start(mch[64:128, :], hloc[0:64, NH - 1 : NH])
        mchn = sb.tile([128, 1], f32)
        nc.vector.tensor_scalar(mchn[:], mch[:], -1.0, None, op0=Alu.mult)

        surv = sb.tile([128, NH], f32)
        nc.scalar.activation(surv[:], hloc[:], Act.Exp, bias=mchn[:], scale=-1.0)

        em = sb.tile([128, NH], f32)
        nc.scalar.activation(em[:], xie[:], Act.Exp)
        steps = sb.tile([128, NH], f32)
        nc.vector.scalar_tensor_tensor(
            steps[:], em[:], -1.0, surv[:], op0=Alu.add, op1=Alu.mult
        )

        # r2 = mask2 / (1 - surv_last), computed on rows 64:128 then shifted down
        t2 = sb.tile([128, 1], f32)
        nc.vector.tensor_scalar(
            t2[64:128, :], surv[64:128, NH - 1 : NH], -1.0, 1.0, op0=Alu.mult, op1=Alu.add
        )
        m2 = sb.tile([128, 1], f32)
        nc.vector.tensor_scalar(m2[64:128, :], t2[64:128, :], EPS, None, op0=Alu.is_ge)
        den = sb.tile([128, 1], f32)
        nc.vector.scalar_tensor_tensor(
            den[64:128, :], t2[64:128, :], -1.0, m2[64:128, :], op0=Alu.add, op1=Alu.mult
        )
        nc.vector.tensor_scalar(den[64:128, :], den[64:128, :], 1.0, None, op0=Alu.add)
        rec = sb.tile([128, 1], f32)
        nc.vector.reciprocal(rec[64:128, :], den[64:128, :])
        r2 = sb.tile([128, 1], f32)
        nc.vector.tensor_tensor(r2[64:128, :], rec[64:128, :], m2[64:128, :], op=Alu.mult)
        nc.sync.dma_start(r2[0:64, :], r2[64:128, :])

        steps_n = sb.tile([128, NH], f32)
        nc.vector.tensor_scalar(steps_n[:], steps[:], r2[:], None, op0=Alu.mult)

        # ---- outputs ----
        nc.sync.dma_start(surv_o[:, 0:NH], surv[0:64, :])
        nc.sync.dma_start(surv_o[:, NH:N], surv[64:128, :])
        nc.sync.dma_start(steps_o[:, 0:NH], steps_n[0:64, :])
        nc.sync.dma_start(steps_o[:, NH:N], steps_n[64:128, :])

    nc.compile()
    return nc


def _get_nc():
    if "nc" not in _cache:
        _cache["nc"] = _build_nc()
    return _cache["nc"]


def _host_prep(c_in, delta_in, c_p, bandwidth):
    import concourse.mybir as mybir

    bf16 = mybir.dt.np(mybir.dt.bfloat16)
    f32 = np.float32

    kv = np.arange(512)
    oh = (c_in[:, kv // V] == (kv % V)[None, :]).T.astype(bf16)  # [512, N]
    oh = np.ascontiguousarray(oh)

    d = delta_in.astype(f32)
    deltab = np.concatenate(
        [np.broadcast_to(d[:NH], (64, NH)), np.broadcast_to(d[NH:], (64, NH))], axis=0
    ).astype(bf16)
    deltab = np.ascontiguousarray(deltab)

    bw = float(np.clip(np.asarray(bandwidth).reshape(-1)[0], 0.1, 10.0))
    mrb = np.full((128, 1), -1.0 / bw, f32)
    scl2 = np.full((128, 1), 2.0 / bw, f32)
    idn = np.eye(64, dtype=f32)

    in_maps = []
    for i in range(NCORES):
        sl = c_p[:, BL * i : BL * (i + 1), :]  # [K, 64, V]
        per = np.ascontiguousarray(sl.transpose(1, 0, 2).reshape(BL, K * V)).astype(f32)
        cp2 = np.concatenate([per, per], axis=0)
        in_maps.append(
            {
                "cp2": np.ascontiguousarray(cp2),
                "ohot": oh,
                "deltab": deltab,
                "mrb": mrb,
                "scl2": scl2,
                "idn": idn,
            }
        )
    return in_maps


def kernel(c_in, delta_in, c_p, bandwidth):
    from concourse.bass_utils import run_bass_kernel_spmd

    c_in = np.asarray(c_in)
    delta_in = np.asarray(delta_in)
    c_p = np.asarray(c_p, dtype=np.float32)
    bandwidth = np.asarray(bandwidth, dtype=np.float32)

    nc = _get_nc()
    in_maps = _host_prep(c_in, delta_in, c_p, bandwidth)
    res = run_bass_kernel_spmd(nc, in_maps, core_ids=list(range(NCORES)))
    results = res.results
    surv = np.concatenate([results[i]["surv_o"] for i in range(NCORES)], axis=0)
    steps = np.concatenate([results[i]["steps_o"] for i in range(NCORES)], axis=0)
    return surv.astype(np.float32), steps.astype(np.float32)
